# revision 4
# baseline (speedup 1.0000x reference)
"""Trainium2 Bass kernel for nn_A2C_DND (A2C agent step with DND kNN retrieval).

Sharding: dict_len (NMEM=65536) is sharded across the 8 NeuronCores (8192
rows of dnd_keys/dnd_vals per core).  Every core computes, for all 1024
cues, the unnormalized softmax-weighted partial value sum and the partial
softmax normalizer over its shard; an AllReduce combines them.  The small
encoder/LSTM/head network is replicated on every core (it overlaps the
collective).  All large matmuls run as float32r (full PE rate, ~1e-4 rel
accuracy).  Outputs are computed transposed on-device and transposed back
on the host.

Softmax is computed without max-subtraction: sims = -sqrt(d2) with d2 ~
N(1024, 55) for this problem's input distribution, so exp(sims) is around
1e-14 -- comfortably inside fp32 range.  h0/c0 are zeros per the problem
spec; the device program exploits that (f-gate and c0/h0 terms dropped).
A full numpy fallback handles the (never occurring) nonzero case.
"""

import os

import numpy as np

import concourse.bass as bass
import concourse.mybir as mybir
from concourse import bacc, tile
from concourse.bass_utils import run_bass_kernel_spmd

B, A, H, DKEY, NMEM = 1024, 4, 512, 512, 65536
OBS, E1, E2 = 9, 64, 128
N_CORES = 8
NM = NMEM // N_CORES  # 8192 rows per core
NCH = NM // 128       # 64 nmem chunks per core

F32 = mybir.dt.float32
F32R = mybir.dt.float32r
AF = mybir.ActivationFunctionType
ALU = mybir.AluOpType

LAST_RESULT = None  # BassKernelResults of the last run (for test.py)

_NC = None  # cached compiled Bacc


def _build():
    nc = bacc.Bacc(
        "TRN2", target_bir_lowering=False, debug=False, num_devices=N_CORES
    )

    def din(name, shape):
        return nc.dram_tensor(name, shape, F32, kind="ExternalInput").ap()

    def dout(name, shape):
        return nc.dram_tensor(name, shape, F32, kind="ExternalOutput").ap()

    cueT = din("cueT", [128, 4, B])       # [k_in, kchunk, b]
    ccb = din("ccb", [1, B])              # ||cue||^2 per b
    keysT = din("keysT", [128, 4, NM])    # [k_in, kchunk, n] (per-core shard)
    kks = din("kks", [128, NCH])          # ||key||^2, [n_in, nchunk]
    vals = din("vals", [NM, H])           # per-core shard, natural layout
    obsT = din("obsT", [OBS, B])
    paprT = din("paprT", [5, B])          # [p_action; p_reward] transposed
    w1T = din("w1T", [OBS, E1])
    b1 = din("b1", [E1, 1])
    w2T = din("w2T", [E1, E2])
    b2 = din("b2", [E2, 1])
    wihTA = din("wihTA", [128, 16, 128])  # w_ih.T[:128, used-cols] per z-chunk
    wihTB = din("wihTB", [5, 16, 128])    # w_ih.T[128:133, used-cols]
    bz = din("bz", [128, 16])             # (b_ih + b_hh)[used-cols]
    awT = din("awT", [128, 4, A])
    ab = din("ab", [A, 1])
    cwT = din("cwT", [128, 4, 1])
    cb = din("cb", [1, 1])

    o_logitsT = dout("o_logitsT", [A, B])
    o_valueT = dout("o_valueT", [1, B])
    o_hT = dout("o_hT", [H, B])
    o_cT = dout("o_cT", [H, B])
    o_featsT = dout("o_featsT", [E2, B])

    # collective bounce buffers (DRAM; outputs must be Shared)
    cc_in = [nc.dram_tensor(f"cc_in{s}", [128, 2048], F32).ap() for s in range(2)]
    cc_out = [
        nc.dram_tensor(f"cc_out{s}", [128, 2048], F32, addr_space="Shared").ap()
        for s in range(2)
    ]
    zc_in = nc.dram_tensor("zc_in", [2, 512], F32).ap()
    zc_out = nc.dram_tensor("zc_out", [2, 512], F32, addr_space="Shared").ap()
    zr_dram = nc.dram_tensor("zr_dram", [2, 512], F32).ap()

    groups = [list(range(N_CORES))]

    def bcast(src_ap, parts):
        # partition-broadcast view of a [1, N] DRAM AP
        return bass.AP(
            tensor=src_ap.tensor, offset=src_ap.offset,
            ap=[[0, parts]] + src_ap.ap[1:],
        )

    with tile.TileContext(nc) as tc:
        with (
            tc.tile_pool(name="consts", bufs=1) as consts,
            tc.tile_pool(name="pdot", bufs=2, space="PSUM") as pdot,
            tc.tile_pool(name="pacc", bufs=1, space="PSUM") as pacc,
        ):
            # ---- resident constants ----
            ct = consts.tile([128, 4, B], F32R)
            nc.sync.dma_start(out=ct[:], in_=cueT.bitcast(F32R))
            cct = consts.tile([128, B], F32)
            nc.sync.dma_start(out=cct[:], in_=bcast(ccb, 128))
            kkt = consts.tile([128, NCH], F32)
            nc.sync.dma_start(out=kkt[:], in_=kks[:])
            ones = consts.tile([128, 1], F32R)
            nc.vector.memset(ones.bitcast(F32), 1.0)
            obst = consts.tile([OBS, B], F32R)
            nc.sync.dma_start(out=obst[:], in_=obsT.bitcast(F32R))
            paprt = consts.tile([5, B], F32R)
            nc.sync.dma_start(out=paprt[:], in_=paprT.bitcast(F32R))
            w1t = consts.tile([OBS, E1], F32R)
            nc.sync.dma_start(out=w1t[:], in_=w1T.bitcast(F32R))
            b1t = consts.tile([E1, 1], F32)
            nc.sync.dma_start(out=b1t[:], in_=b1[:])
            w2t = consts.tile([E1, E2], F32R)
            nc.sync.dma_start(out=w2t[:], in_=w2T.bitcast(F32R))
            b2t = consts.tile([E2, 1], F32)
            nc.sync.dma_start(out=b2t[:], in_=b2[:])
            awt = consts.tile([128, 4, A], F32R)
            nc.sync.dma_start(out=awt[:], in_=awT.bitcast(F32R))
            abt = consts.tile([A, 1], F32)
            nc.sync.dma_start(out=abt[:], in_=ab[:])
            cwt = consts.tile([128, 4, 1], F32R)
            nc.sync.dma_start(out=cwt[:], in_=cwT.bitcast(F32R))
            cbt = consts.tile([1, 1], F32)
            nc.sync.dma_start(out=cbt[:], in_=cb[:])

            # ---- encoder MLP (replicated, tiny) ----
            f1 = consts.tile([E1, B], F32R)
            feats = consts.tile([E2, B], F32R)
            for s in range(2):
                sl = slice(s * 512, (s + 1) * 512)
                p1 = pdot.tile([E1, 512], F32, tag="dot")
                nc.tensor.matmul(p1[:], lhsT=w1t[:], rhs=obst[:, sl],
                                 start=True, stop=True)
                nc.scalar.activation(out=f1[:, sl], in_=p1[:], func=AF.Relu,
                                     bias=b1t[:])
                p2 = pdot.tile([E2, 512], F32, tag="dot")
                nc.tensor.matmul(p2[:], lhsT=w2t[:], rhs=f1[:, sl],
                                 start=True, stop=True)
                nc.scalar.activation(out=feats[:, sl], in_=p2[:], func=AF.Relu,
                                     bias=b2t[:])
            nc.sync.dma_start(out=o_featsT[:], in_=feats.bitcast(F32))

            # ---- DND retrieval over the local shard ----
            with (
                tc.tile_pool(name="keys", bufs=1) as keys_pool,
                tc.tile_pool(name="vstream", bufs=4) as vstream,
                tc.tile_pool(name="estream", bufs=4) as estream,
                tc.tile_pool(name="stage", bufs=1) as stage_pool,
            ):
                kt = []
                for i in range(8):
                    t = keys_pool.tile([128, 4, 1024], F32R, tag=f"kt{i}")
                    nc.sync.dma_start(
                        out=t[:],
                        in_=keysT[:, :, i * 1024:(i + 1) * 1024].bitcast(F32R),
                    )
                    kt.append(t)

                for s in range(2):
                    bsl = slice(s * 512, (s + 1) * 512)
                    pv = [pacc.tile([128, 512], F32, tag=f"pv{h}",
                                    name=f"pv{h}") for h in range(4)]
                    za = pacc.tile([1, 512], F32, tag="za")
                    for j in range(NCH):
                        pd = pdot.tile([128, 512], F32, tag="dot")
                        kslab = kt[j // 8]
                        off = (j % 8) * 128
                        for k in range(4):
                            nc.tensor.matmul(
                                pd[:], lhsT=kslab[:, k, off:off + 128],
                                rhs=ct[:, k, bsl],
                                start=(k == 0), stop=(k == 3),
                            )
                        # d2 = -2*dot + ||cue||^2 + ||key||^2 ; s = sqrt(d2)
                        nc.vector.scalar_tensor_tensor(
                            out=pd[:], in0=pd[:], scalar=-2.0, in1=cct[:, bsl],
                            op0=ALU.mult, op1=ALU.add,
                        )
                        nc.scalar.activation(out=pd[:], in_=pd[:], func=AF.Sqrt,
                                             bias=kkt[:, j:j + 1])
                        et = estream.tile([128, 512], F32R, tag="et")
                        nc.scalar.activation(out=et[:], in_=pd[:], func=AF.Exp,
                                             scale=-1.0)
                        vt = vstream.tile([128, 512], F32R, tag="vt")
                        nc.sync.dma_start(
                            out=vt[:],
                            in_=vals[j * 128:(j + 1) * 128, :].bitcast(F32R),
                        )
                        for h in range(4):
                            nc.tensor.matmul(
                                pv[h][:], lhsT=vt[:, h * 128:(h + 1) * 128],
                                rhs=et[:],
                                start=(j == 0), stop=(j == NCH - 1),
                            )
                        nc.tensor.matmul(za[:], lhsT=ones[:], rhs=et[:],
                                         start=(j == 0), stop=(j == NCH - 1))
                    # flush partials -> DRAM -> AllReduce
                    st = stage_pool.tile([128, 2048], F32, tag="mst")
                    for h in range(4):
                        nc.vector.tensor_copy(st[:, h * 512:(h + 1) * 512],
                                              pv[h][:])
                    zst = stage_pool.tile([1, 512], F32, tag="zst")
                    nc.vector.tensor_copy(zst[:], za[:])
                    nc.sync.dma_start(out=cc_in[s][:], in_=st[:])
                    nc.sync.dma_start(out=zc_in[s:s + 1, :], in_=zst[:])
                    nc.gpsimd.collective_compute(
                        "AllReduce", ALU.add, replica_groups=groups,
                        ins=[cc_in[s].opt()], outs=[cc_out[s].opt()],
                    )
                nc.gpsimd.collective_compute(
                    "AllReduce", ALU.add, replica_groups=groups,
                    ins=[zc_in.opt()], outs=[zc_out.opt()],
                )

            # ---- LSTM gates (overlap the collectives) ----
            with tc.tile_pool(name="gates", bufs=1) as gates:
                gfuncs = [AF.Sigmoid, AF.Tanh, AF.Sigmoid, AF.Sigmoid]  # i g o r
                gt = [gates.tile([128, 4, B], F32, tag=f"gate{g}",
                                 name=f"gate{g}") for g in range(4)]
                with tc.tile_pool(name="wpool", bufs=1) as wpool:
                    wiha = wpool.tile([128, 16, 128], F32R)
                    nc.sync.dma_start(out=wiha[:], in_=wihTA.bitcast(F32R))
                    wihb = wpool.tile([5, 16, 128], F32R)
                    nc.sync.dma_start(out=wihb[:], in_=wihTB.bitcast(F32R))
                    bzt = wpool.tile([128, 16], F32)
                    nc.sync.dma_start(out=bzt[:], in_=bz[:])

                    for u in range(16):
                        gi, hc = u // 4, u % 4
                        for s in range(2):
                            bsl = slice(s * 512, (s + 1) * 512)
                            pz = pdot.tile([128, 512], F32, tag="dot")
                            nc.tensor.matmul(pz[:], lhsT=wiha[:, u, :],
                                             rhs=feats[:, bsl],
                                             start=True, stop=False)
                            nc.tensor.matmul(pz[:], lhsT=wihb[:, u, :],
                                             rhs=paprt[:, bsl],
                                             start=False, stop=True)
                            nc.scalar.activation(out=gt[gi][:, hc, bsl],
                                                 in_=pz[:], func=gfuncs[gi],
                                                 bias=bzt[:, u:u + 1])

                # ---- combine: m_t = m_sum / z_sum ; LSTM cell ; heads ----
                tmp_cm = tc.tile_pool(name="tmp", bufs=2)
                tmp = tmp_cm.__enter__()
                m_sum = gates.tile([128, 4, B], F32)
                for s in range(2):
                    for h in range(4):
                        nc.sync.dma_start(
                            out=m_sum[:, h, s * 512:(s + 1) * 512],
                            in_=cc_out[s][:, h * 512:(h + 1) * 512],
                        )
                zt = gates.tile([2, 512], F32)
                nc.sync.dma_start(out=zt[:], in_=zc_out[:])
                zr = gates.tile([2, 512], F32)
                nc.vector.reciprocal(zr[:], zt[:])
                nc.sync.dma_start(out=zr_dram[:], in_=zr[:])
                rb = gates.tile([128, B], F32)
                for s in range(2):
                    src = zr_dram[s:s + 1, :]
                    nc.sync.dma_start(out=rb[:, s * 512:(s + 1) * 512],
                                      in_=bcast(src, 128))

                hTr = gates.tile([128, 4, B], F32R)
                ti, tg, to, tr = gt
                for h in range(4):
                    mn = m_sum[:, h, :]
                    nc.vector.tensor_mul(mn, mn, rb[:])        # normalize
                    nc.vector.tensor_mul(mn, mn, tr[:, h, :])  # r * m_t
                    t1 = tmp.tile([128, B], F32, tag="t1")
                    nc.vector.tensor_mul(t1[:], ti[:, h, :], tg[:, h, :])
                    nc.vector.tensor_add(t1[:], t1[:], mn)     # c_new chunk
                    nc.sync.dma_start(out=o_cT[h * 128:(h + 1) * 128, :],
                                      in_=t1[:])
                    th = tmp.tile([128, B], F32, tag="th")
                    nc.scalar.activation(out=th[:], in_=t1[:], func=AF.Tanh)
                    nc.vector.tensor_mul(th[:], to[:, h, :], th[:])  # h_new
                    nc.sync.dma_start(out=o_hT[h * 128:(h + 1) * 128, :],
                                      in_=th[:])
                    nc.scalar.activation(out=hTr[:, h, :], in_=th[:],
                                         func=AF.Copy)

                lg = gates.tile([A, B], F32)
                vl = gates.tile([1, B], F32)
                for s in range(2):
                    bsl = slice(s * 512, (s + 1) * 512)
                    pl = pdot.tile([A, 512], F32, tag="dot")
                    for h in range(4):
                        nc.tensor.matmul(pl[:], lhsT=awt[:, h, :],
                                         rhs=hTr[:, h, bsl],
                                         start=(h == 0), stop=(h == 3))
                    nc.scalar.activation(out=lg[:, bsl], in_=pl[:],
                                         func=AF.Identity, bias=abt[:])
                    pv2 = pdot.tile([1, 512], F32, tag="dot")
                    for h in range(4):
                        nc.tensor.matmul(pv2[:], lhsT=cwt[:, h, :],
                                         rhs=hTr[:, h, bsl],
                                         start=(h == 0), stop=(h == 3))
                    nc.scalar.activation(out=vl[:, bsl], in_=pv2[:],
                                         func=AF.Identity, bias=cbt[:])
                nc.sync.dma_start(out=o_logitsT[:], in_=lg[:])
                nc.sync.dma_start(out=o_valueT[:], in_=vl[:])
                tmp_cm.__exit__(None, None, None)

    nc.compile()
    return nc


def _np_reference(obs, p_action, p_reward, h0, c0, cue,
                  enc_w1, enc_b1, enc_w2, enc_b2,
                  w_ih, w_hh, b_ih, b_hh,
                  actor_w, actor_b, critic_w, critic_b,
                  dnd_keys, dnd_vals):
    # exact numpy fallback (used only if h0/c0 are nonzero)
    def sigmoid(x):
        return 1.0 / (1.0 + np.exp(-x))

    relu = lambda x: np.maximum(x, 0.0)
    feats = relu(relu(obs @ enc_w1.T + enc_b1) @ enc_w2.T + enc_b2)
    x_t = np.concatenate([feats, p_action, p_reward], axis=-1)
    d2 = ((cue * cue).sum(-1, keepdims=True) - 2.0 * (cue @ dnd_keys.T)
          + (dnd_keys * dnd_keys).sum(-1))
    sims = -np.sqrt(np.maximum(d2, 1e-12))
    sims = sims - sims.max(-1, keepdims=True)
    w = np.exp(sims)
    w /= w.sum(-1, keepdims=True)
    m_t = w @ dnd_vals
    h, c = h0[0], c0[0]
    z = x_t @ w_ih.T + h @ w_hh.T + b_ih + b_hh
    i_g = sigmoid(z[:, 0 * H:1 * H])
    f_g = sigmoid(z[:, 1 * H:2 * H])
    g_g = np.tanh(z[:, 2 * H:3 * H])
    o_g = sigmoid(z[:, 3 * H:4 * H])
    r_g = sigmoid(z[:, 4 * H:5 * H])
    c_new = f_g * c + i_g * g_g + r_g * m_t
    h_new = o_g * np.tanh(c_new)
    action_logits = h_new @ actor_w.T + actor_b
    value_estimate = h_new @ critic_w.T + critic_b
    return (action_logits.astype(np.float32),
            value_estimate.astype(np.float32),
            h_new[None].astype(np.float32), c_new[None].astype(np.float32),
            feats.astype(np.float32))


def kernel(obs, p_action, p_reward, h0, c0, cue,
           enc_w1, enc_b1, enc_w2, enc_b2,
           w_ih, w_hh, b_ih, b_hh,
           actor_w, actor_b, critic_w, critic_b,
           dnd_keys, dnd_vals):
    global _NC, LAST_RESULT
    f = lambda x: np.ascontiguousarray(np.asarray(x, dtype=np.float32))
    obs, p_action, p_reward = f(obs), f(p_action), f(p_reward)
    h0, c0, cue = f(h0), f(c0), f(cue)
    enc_w1, enc_b1, enc_w2, enc_b2 = f(enc_w1), f(enc_b1), f(enc_w2), f(enc_b2)
    w_ih, w_hh, b_ih, b_hh = f(w_ih), f(w_hh), f(b_ih), f(b_hh)
    actor_w, actor_b, critic_w, critic_b = f(actor_w), f(actor_b), f(critic_w), f(critic_b)
    dnd_keys, dnd_vals = f(dnd_keys), f(dnd_vals)

    if np.any(h0) or np.any(c0):
        return _np_reference(obs, p_action, p_reward, h0, c0, cue,
                             enc_w1, enc_b1, enc_w2, enc_b2,
                             w_ih, w_hh, b_ih, b_hh,
                             actor_w, actor_b, critic_w, critic_b,
                             dnd_keys, dnd_vals)

    if _NC is None:
        _NC = _build()

    cueT_n = np.ascontiguousarray(cue.T.reshape(4, 128, B).transpose(1, 0, 2))
    ccb_n = np.ascontiguousarray((cue * cue).sum(1)[None, :])
    obsT_n = np.ascontiguousarray(obs.T)
    paprT_n = np.ascontiguousarray(
        np.concatenate([p_action.T, p_reward.T], axis=0))
    w1T_n = np.ascontiguousarray(enc_w1.T)
    b1_n = np.ascontiguousarray(enc_b1[:, None])
    w2T_n = np.ascontiguousarray(enc_w2.T)
    b2_n = np.ascontiguousarray(enc_b2[:, None])
    cols = np.r_[0:512, 1024:2560]
    wihT = w_ih.T
    wihTA_n = np.ascontiguousarray(wihT[:128, cols].reshape(128, 16, 128))
    wihTB_n = np.ascontiguousarray(wihT[128:133, cols].reshape(5, 16, 128))
    bz_n = np.ascontiguousarray((b_ih + b_hh)[cols].reshape(16, 128).T)
    awT_n = np.ascontiguousarray(actor_w.T.reshape(4, 128, A).transpose(1, 0, 2))
    ab_n = np.ascontiguousarray(actor_b[:, None])
    cwT_n = np.ascontiguousarray(critic_w.T.reshape(4, 128, 1).transpose(1, 0, 2))
    cb_n = np.ascontiguousarray(critic_b[:, None])

    shared = {
        "cueT": cueT_n, "ccb": ccb_n, "obsT": obsT_n, "paprT": paprT_n,
        "w1T": w1T_n, "b1": b1_n, "w2T": w2T_n, "b2": b2_n,
        "wihTA": wihTA_n, "wihTB": wihTB_n, "bz": bz_n,
        "awT": awT_n, "ab": ab_n, "cwT": cwT_n, "cb": cb_n,
    }
    in_maps = []
    for i in range(N_CORES):
        keys_i = dnd_keys[i * NM:(i + 1) * NM]
        m = dict(shared)
        m["keysT"] = np.ascontiguousarray(
            keys_i.T.reshape(4, 128, NM).transpose(1, 0, 2))
        m["kks"] = np.ascontiguousarray(
            (keys_i * keys_i).sum(1).reshape(NCH, 128).T)
        m["vals"] = np.ascontiguousarray(dnd_vals[i * NM:(i + 1) * NM])
        in_maps.append(m)

    trace = bool(os.environ.get("BASS_TRACE"))
    res = run_bass_kernel_spmd(_NC, in_maps, list(range(N_CORES)), trace=trace)
    LAST_RESULT = res
    r = res.results[0]
    action_logits = np.ascontiguousarray(r["o_logitsT"].T)
    value_estimate = np.ascontiguousarray(r["o_valueT"].T)
    h_new = np.ascontiguousarray(r["o_hT"].T)[None]
    c_new = np.ascontiguousarray(r["o_cT"].T)[None]
    feats = np.ascontiguousarray(r["o_featsT"].T)
    return (action_logits, value_estimate, h_new, c_new, feats)


# revision 5
# speedup vs baseline: 1.1497x; 1.1497x over previous
"""Trainium2 Bass kernel for nn_A2C_DND (A2C agent step with DND kNN retrieval).

Sharding: dict_len (NMEM=65536) is sharded across the 8 NeuronCores (8192
rows of dnd_keys/dnd_vals per core).  Every core computes, for all 1024
cues, the unnormalized softmax-weighted partial value sum and the partial
softmax normalizer over its shard; an AllReduce combines them.  The small
encoder/LSTM/head network is replicated on every core (it overlaps the
collective).  All large matmuls run as float32r (full PE rate, ~1e-4 rel
accuracy).  Outputs are computed transposed on-device and transposed back
on the host.

Softmax is computed without max-subtraction: sims = -sqrt(d2) with d2 ~
N(1024, 55) for this problem's input distribution, so exp(sims) is around
1e-14 -- comfortably inside fp32 range.  h0/c0 are zeros per the problem
spec; the device program exploits that (f-gate and c0/h0 terms dropped).
A full numpy fallback handles the (never occurring) nonzero case.
"""

import os

import numpy as np

import concourse.bass as bass
import concourse.mybir as mybir
from concourse import bacc, tile
from concourse.bass_utils import run_bass_kernel_spmd

B, A, H, DKEY, NMEM = 1024, 4, 512, 512, 65536
OBS, E1, E2 = 9, 64, 128
N_CORES = 8
NM = NMEM // N_CORES  # 8192 rows per core
NCH = NM // 128       # 64 nmem chunks per core

F32 = mybir.dt.float32
F32R = mybir.dt.float32r
AF = mybir.ActivationFunctionType
ALU = mybir.AluOpType

LAST_RESULT = None  # BassKernelResults of the last run (for test.py)

_NC = None  # cached compiled Bacc


def _build():
    nc = bacc.Bacc(
        "TRN2", target_bir_lowering=False, debug=False, num_devices=N_CORES
    )

    def din(name, shape):
        return nc.dram_tensor(name, shape, F32, kind="ExternalInput").ap()

    def dout(name, shape):
        return nc.dram_tensor(name, shape, F32, kind="ExternalOutput").ap()

    cueT = din("cueT", [128, 4, B])       # [k_in, kchunk, b]
    ccb = din("ccb", [1, B])              # ||cue||^2 per b
    keysT = din("keysT", [128, 4, NM])    # [k_in, kchunk, n] (per-core shard)
    kks = din("kks", [128, NCH])          # ||key||^2, [n_in, nchunk]
    vals = din("vals", [NM, H])           # per-core shard, natural layout
    obsT = din("obsT", [OBS, B])
    paprT = din("paprT", [5, B])          # [p_action; p_reward] transposed
    w1T = din("w1T", [OBS, E1])
    b1 = din("b1", [E1, 1])
    w2T = din("w2T", [E1, E2])
    b2 = din("b2", [E2, 1])
    wihTA = din("wihTA", [128, 16, 128])  # w_ih.T[:128, used-cols] per z-chunk
    wihTB = din("wihTB", [5, 16, 128])    # w_ih.T[128:133, used-cols]
    bz = din("bz", [128, 16])             # (b_ih + b_hh)[used-cols]
    awT = din("awT", [128, 4, A])
    ab = din("ab", [A, 1])
    cwT = din("cwT", [128, 4, 1])
    cb = din("cb", [1, 1])

    o_logitsT = dout("o_logitsT", [A, B])
    o_valueT = dout("o_valueT", [1, B])
    o_hT = dout("o_hT", [H, B])
    o_cT = dout("o_cT", [H, B])
    o_featsT = dout("o_featsT", [E2, B])

    # collective bounce buffers (DRAM; outputs must be Shared)
    cc_in = [nc.dram_tensor(f"cc_in{s}", [128, 2048], F32).ap() for s in range(2)]
    cc_out = [
        nc.dram_tensor(f"cc_out{s}", [128, 2048], F32, addr_space="Shared").ap()
        for s in range(2)
    ]
    zc_in = nc.dram_tensor("zc_in", [2, 512], F32).ap()
    zc_out = nc.dram_tensor("zc_out", [2, 512], F32, addr_space="Shared").ap()
    zr_dram = nc.dram_tensor("zr_dram", [2, 512], F32).ap()

    groups = [list(range(N_CORES))]

    def bcast(src_ap, parts):
        # partition-broadcast view of a [1, N] DRAM AP
        return bass.AP(
            tensor=src_ap.tensor, offset=src_ap.offset,
            ap=[[0, parts]] + src_ap.ap[1:],
        )

    with tile.TileContext(nc) as tc:
        with (
            tc.tile_pool(name="consts", bufs=1) as consts,
            tc.tile_pool(name="pdot", bufs=3, space="PSUM") as pdot,
            tc.tile_pool(name="pacc", bufs=1, space="PSUM") as pacc,
        ):
            # ---- resident constants ----
            ct = consts.tile([128, 4, B], F32R)
            nc.sync.dma_start(out=ct[:], in_=cueT.bitcast(F32R))
            cct = consts.tile([128, B], F32)
            nc.sync.dma_start(out=cct[:], in_=bcast(ccb, 128))
            kkt = consts.tile([128, NCH], F32)
            nc.sync.dma_start(out=kkt[:], in_=kks[:])
            ones = consts.tile([128, 1], F32R)
            nc.vector.memset(ones.bitcast(F32), 1.0)
            obst = consts.tile([OBS, B], F32R)
            nc.sync.dma_start(out=obst[:], in_=obsT.bitcast(F32R))
            paprt = consts.tile([5, B], F32R)
            nc.sync.dma_start(out=paprt[:], in_=paprT.bitcast(F32R))
            w1t = consts.tile([OBS, E1], F32R)
            nc.sync.dma_start(out=w1t[:], in_=w1T.bitcast(F32R))
            b1t = consts.tile([E1, 1], F32)
            nc.sync.dma_start(out=b1t[:], in_=b1[:])
            w2t = consts.tile([E1, E2], F32R)
            nc.sync.dma_start(out=w2t[:], in_=w2T.bitcast(F32R))
            b2t = consts.tile([E2, 1], F32)
            nc.sync.dma_start(out=b2t[:], in_=b2[:])
            awt = consts.tile([128, 4, A], F32R)
            nc.sync.dma_start(out=awt[:], in_=awT.bitcast(F32R))
            abt = consts.tile([A, 1], F32)
            nc.sync.dma_start(out=abt[:], in_=ab[:])
            cwt = consts.tile([128, 4, 1], F32R)
            nc.sync.dma_start(out=cwt[:], in_=cwT.bitcast(F32R))
            cbt = consts.tile([1, 1], F32)
            nc.sync.dma_start(out=cbt[:], in_=cb[:])

            # ---- encoder MLP (replicated, tiny) ----
            f1 = consts.tile([E1, B], F32R)
            feats = consts.tile([E2, B], F32R)
            for s in range(2):
                sl = slice(s * 512, (s + 1) * 512)
                p1 = pdot.tile([E1, 512], F32, tag="dot")
                nc.tensor.matmul(p1[:], lhsT=w1t[:], rhs=obst[:, sl],
                                 start=True, stop=True)
                nc.scalar.activation(out=f1[:, sl], in_=p1[:], func=AF.Relu,
                                     bias=b1t[:])
                p2 = pdot.tile([E2, 512], F32, tag="dot")
                nc.tensor.matmul(p2[:], lhsT=w2t[:], rhs=f1[:, sl],
                                 start=True, stop=True)
                nc.scalar.activation(out=feats[:, sl], in_=p2[:], func=AF.Relu,
                                     bias=b2t[:])
            nc.sync.dma_start(out=o_featsT[:], in_=feats.bitcast(F32))

            # ---- DND retrieval over the local shard ----
            with (
                tc.tile_pool(name="keys", bufs=1) as keys_pool,
                tc.tile_pool(name="vstream", bufs=4) as vstream,
                tc.tile_pool(name="estream", bufs=4) as estream,
                tc.tile_pool(name="stage", bufs=1) as stage_pool,
            ):
                kt = []
                for i in range(8):
                    t = keys_pool.tile([128, 4, 1024], F32R, tag=f"kt{i}")
                    nc.sync.dma_start(
                        out=t[:],
                        in_=keysT[:, :, i * 1024:(i + 1) * 1024].bitcast(F32R),
                    )
                    kt.append(t)

                for s in range(2):
                    bsl = slice(s * 512, (s + 1) * 512)
                    pv = [pacc.tile([128, 512], F32, tag=f"pv{h}",
                                    name=f"pv{h}") for h in range(4)]
                    za = pacc.tile([1, 512], F32, tag="za")
                    for j in range(NCH):
                        pd = pdot.tile([128, 512], F32, tag="dot")
                        kslab = kt[j // 8]
                        off = (j % 8) * 128
                        for k in range(4):
                            nc.tensor.matmul(
                                pd[:], lhsT=kslab[:, k, off:off + 128],
                                rhs=ct[:, k, bsl],
                                start=(k == 0), stop=(k == 3),
                            )
                        # d2 = -2*dot + ||cue||^2 + ||key||^2
                        # exp(-sqrt(d2)) = exp(-exp(0.5*ln(d2))): ln and exp
                        # share one ACT table set (no per-tile table swaps)
                        nc.vector.scalar_tensor_tensor(
                            out=pd[:], in0=pd[:], scalar=-2.0, in1=cct[:, bsl],
                            op0=ALU.mult, op1=ALU.add,
                        )
                        nc.scalar.activation(out=pd[:], in_=pd[:], func=AF.Ln,
                                             bias=kkt[:, j:j + 1])
                        nc.scalar.activation(out=pd[:], in_=pd[:], func=AF.Exp,
                                             scale=0.5)
                        et = estream.tile([128, 512], F32R, tag="et")
                        nc.scalar.activation(out=et[:], in_=pd[:], func=AF.Exp,
                                             scale=-1.0)
                        vt = vstream.tile([128, 512], F32R, tag="vt")
                        nc.sync.dma_start(
                            out=vt[:],
                            in_=vals[j * 128:(j + 1) * 128, :].bitcast(F32R),
                        )
                        for h in range(4):
                            nc.tensor.matmul(
                                pv[h][:], lhsT=vt[:, h * 128:(h + 1) * 128],
                                rhs=et[:],
                                start=(j == 0), stop=(j == NCH - 1),
                            )
                        nc.tensor.matmul(za[:], lhsT=ones[:], rhs=et[:],
                                         start=(j == 0), stop=(j == NCH - 1))
                    # flush partials -> DRAM -> AllReduce
                    st = stage_pool.tile([128, 2048], F32, tag="mst")
                    for h in range(4):
                        nc.vector.tensor_copy(st[:, h * 512:(h + 1) * 512],
                                              pv[h][:])
                    zst = stage_pool.tile([1, 512], F32, tag="zst")
                    nc.vector.tensor_copy(zst[:], za[:])
                    nc.sync.dma_start(out=cc_in[s][:], in_=st[:])
                    nc.sync.dma_start(out=zc_in[s:s + 1, :], in_=zst[:])
                    nc.gpsimd.collective_compute(
                        "AllReduce", ALU.add, replica_groups=groups,
                        ins=[cc_in[s].opt()], outs=[cc_out[s].opt()],
                    )
                nc.gpsimd.collective_compute(
                    "AllReduce", ALU.add, replica_groups=groups,
                    ins=[zc_in.opt()], outs=[zc_out.opt()],
                )

            # ---- LSTM gates (overlap the collectives) ----
            with tc.tile_pool(name="gates", bufs=1) as gates:
                gfuncs = [AF.Sigmoid, AF.Tanh, AF.Sigmoid, AF.Sigmoid]  # i g o r
                gt = [gates.tile([128, 4, B], F32, tag=f"gate{g}",
                                 name=f"gate{g}") for g in range(4)]
                with tc.tile_pool(name="wpool", bufs=1) as wpool:
                    wiha = wpool.tile([128, 16, 128], F32R)
                    nc.sync.dma_start(out=wiha[:], in_=wihTA.bitcast(F32R))
                    wihb = wpool.tile([5, 16, 128], F32R)
                    nc.sync.dma_start(out=wihb[:], in_=wihTB.bitcast(F32R))
                    bzt = wpool.tile([128, 16], F32)
                    nc.sync.dma_start(out=bzt[:], in_=bz[:])

                    for u in range(16):
                        gi, hc = u // 4, u % 4
                        for s in range(2):
                            bsl = slice(s * 512, (s + 1) * 512)
                            pz = pdot.tile([128, 512], F32, tag="dot")
                            nc.tensor.matmul(pz[:], lhsT=wiha[:, u, :],
                                             rhs=feats[:, bsl],
                                             start=True, stop=False)
                            nc.tensor.matmul(pz[:], lhsT=wihb[:, u, :],
                                             rhs=paprt[:, bsl],
                                             start=False, stop=True)
                            nc.scalar.activation(out=gt[gi][:, hc, bsl],
                                                 in_=pz[:], func=gfuncs[gi],
                                                 bias=bzt[:, u:u + 1])

                # ---- combine: m_t = m_sum / z_sum ; LSTM cell ; heads ----
                tmp_cm = tc.tile_pool(name="tmp", bufs=2)
                tmp = tmp_cm.__enter__()
                m_sum = gates.tile([128, 4, B], F32)
                for s in range(2):
                    for h in range(4):
                        nc.sync.dma_start(
                            out=m_sum[:, h, s * 512:(s + 1) * 512],
                            in_=cc_out[s][:, h * 512:(h + 1) * 512],
                        )
                zt = gates.tile([2, 512], F32)
                nc.sync.dma_start(out=zt[:], in_=zc_out[:])
                zr = gates.tile([2, 512], F32)
                nc.vector.reciprocal(zr[:], zt[:])
                nc.sync.dma_start(out=zr_dram[:], in_=zr[:])
                rb = gates.tile([128, B], F32)
                for s in range(2):
                    src = zr_dram[s:s + 1, :]
                    nc.sync.dma_start(out=rb[:, s * 512:(s + 1) * 512],
                                      in_=bcast(src, 128))

                hTr = gates.tile([128, 4, B], F32R)
                ti, tg, to, tr = gt
                for h in range(4):
                    mn = m_sum[:, h, :]
                    nc.vector.tensor_mul(mn, mn, rb[:])        # normalize
                    nc.vector.tensor_mul(mn, mn, tr[:, h, :])  # r * m_t
                    t1 = tmp.tile([128, B], F32, tag="t1")
                    nc.vector.tensor_mul(t1[:], ti[:, h, :], tg[:, h, :])
                    nc.vector.tensor_add(t1[:], t1[:], mn)     # c_new chunk
                    nc.sync.dma_start(out=o_cT[h * 128:(h + 1) * 128, :],
                                      in_=t1[:])
                    th = tmp.tile([128, B], F32, tag="th")
                    nc.scalar.activation(out=th[:], in_=t1[:], func=AF.Tanh)
                    nc.vector.tensor_mul(th[:], to[:, h, :], th[:])  # h_new
                    nc.sync.dma_start(out=o_hT[h * 128:(h + 1) * 128, :],
                                      in_=th[:])
                    nc.scalar.activation(out=hTr[:, h, :], in_=th[:],
                                         func=AF.Copy)

                lg = gates.tile([A, B], F32)
                vl = gates.tile([1, B], F32)
                for s in range(2):
                    bsl = slice(s * 512, (s + 1) * 512)
                    pl = pdot.tile([A, 512], F32, tag="dot")
                    for h in range(4):
                        nc.tensor.matmul(pl[:], lhsT=awt[:, h, :],
                                         rhs=hTr[:, h, bsl],
                                         start=(h == 0), stop=(h == 3))
                    nc.scalar.activation(out=lg[:, bsl], in_=pl[:],
                                         func=AF.Identity, bias=abt[:])
                    pv2 = pdot.tile([1, 512], F32, tag="dot")
                    for h in range(4):
                        nc.tensor.matmul(pv2[:], lhsT=cwt[:, h, :],
                                         rhs=hTr[:, h, bsl],
                                         start=(h == 0), stop=(h == 3))
                    nc.scalar.activation(out=vl[:, bsl], in_=pv2[:],
                                         func=AF.Identity, bias=cbt[:])
                nc.sync.dma_start(out=o_logitsT[:], in_=lg[:])
                nc.sync.dma_start(out=o_valueT[:], in_=vl[:])
                tmp_cm.__exit__(None, None, None)

    nc.compile()
    return nc


def _np_reference(obs, p_action, p_reward, h0, c0, cue,
                  enc_w1, enc_b1, enc_w2, enc_b2,
                  w_ih, w_hh, b_ih, b_hh,
                  actor_w, actor_b, critic_w, critic_b,
                  dnd_keys, dnd_vals):
    # exact numpy fallback (used only if h0/c0 are nonzero)
    def sigmoid(x):
        return 1.0 / (1.0 + np.exp(-x))

    relu = lambda x: np.maximum(x, 0.0)
    feats = relu(relu(obs @ enc_w1.T + enc_b1) @ enc_w2.T + enc_b2)
    x_t = np.concatenate([feats, p_action, p_reward], axis=-1)
    d2 = ((cue * cue).sum(-1, keepdims=True) - 2.0 * (cue @ dnd_keys.T)
          + (dnd_keys * dnd_keys).sum(-1))
    sims = -np.sqrt(np.maximum(d2, 1e-12))
    sims = sims - sims.max(-1, keepdims=True)
    w = np.exp(sims)
    w /= w.sum(-1, keepdims=True)
    m_t = w @ dnd_vals
    h, c = h0[0], c0[0]
    z = x_t @ w_ih.T + h @ w_hh.T + b_ih + b_hh
    i_g = sigmoid(z[:, 0 * H:1 * H])
    f_g = sigmoid(z[:, 1 * H:2 * H])
    g_g = np.tanh(z[:, 2 * H:3 * H])
    o_g = sigmoid(z[:, 3 * H:4 * H])
    r_g = sigmoid(z[:, 4 * H:5 * H])
    c_new = f_g * c + i_g * g_g + r_g * m_t
    h_new = o_g * np.tanh(c_new)
    action_logits = h_new @ actor_w.T + actor_b
    value_estimate = h_new @ critic_w.T + critic_b
    return (action_logits.astype(np.float32),
            value_estimate.astype(np.float32),
            h_new[None].astype(np.float32), c_new[None].astype(np.float32),
            feats.astype(np.float32))


def kernel(obs, p_action, p_reward, h0, c0, cue,
           enc_w1, enc_b1, enc_w2, enc_b2,
           w_ih, w_hh, b_ih, b_hh,
           actor_w, actor_b, critic_w, critic_b,
           dnd_keys, dnd_vals):
    global _NC, LAST_RESULT
    f = lambda x: np.ascontiguousarray(np.asarray(x, dtype=np.float32))
    obs, p_action, p_reward = f(obs), f(p_action), f(p_reward)
    h0, c0, cue = f(h0), f(c0), f(cue)
    enc_w1, enc_b1, enc_w2, enc_b2 = f(enc_w1), f(enc_b1), f(enc_w2), f(enc_b2)
    w_ih, w_hh, b_ih, b_hh = f(w_ih), f(w_hh), f(b_ih), f(b_hh)
    actor_w, actor_b, critic_w, critic_b = f(actor_w), f(actor_b), f(critic_w), f(critic_b)
    dnd_keys, dnd_vals = f(dnd_keys), f(dnd_vals)

    if np.any(h0) or np.any(c0):
        return _np_reference(obs, p_action, p_reward, h0, c0, cue,
                             enc_w1, enc_b1, enc_w2, enc_b2,
                             w_ih, w_hh, b_ih, b_hh,
                             actor_w, actor_b, critic_w, critic_b,
                             dnd_keys, dnd_vals)

    if _NC is None:
        _NC = _build()

    cueT_n = np.ascontiguousarray(cue.T.reshape(4, 128, B).transpose(1, 0, 2))
    ccb_n = np.ascontiguousarray((cue * cue).sum(1)[None, :])
    obsT_n = np.ascontiguousarray(obs.T)
    paprT_n = np.ascontiguousarray(
        np.concatenate([p_action.T, p_reward.T], axis=0))
    w1T_n = np.ascontiguousarray(enc_w1.T)
    b1_n = np.ascontiguousarray(enc_b1[:, None])
    w2T_n = np.ascontiguousarray(enc_w2.T)
    b2_n = np.ascontiguousarray(enc_b2[:, None])
    cols = np.r_[0:512, 1024:2560]
    wihT = w_ih.T
    wihTA_n = np.ascontiguousarray(wihT[:128, cols].reshape(128, 16, 128))
    wihTB_n = np.ascontiguousarray(wihT[128:133, cols].reshape(5, 16, 128))
    bz_n = np.ascontiguousarray((b_ih + b_hh)[cols].reshape(16, 128).T)
    awT_n = np.ascontiguousarray(actor_w.T.reshape(4, 128, A).transpose(1, 0, 2))
    ab_n = np.ascontiguousarray(actor_b[:, None])
    cwT_n = np.ascontiguousarray(critic_w.T.reshape(4, 128, 1).transpose(1, 0, 2))
    cb_n = np.ascontiguousarray(critic_b[:, None])

    shared = {
        "cueT": cueT_n, "ccb": ccb_n, "obsT": obsT_n, "paprT": paprT_n,
        "w1T": w1T_n, "b1": b1_n, "w2T": w2T_n, "b2": b2_n,
        "wihTA": wihTA_n, "wihTB": wihTB_n, "bz": bz_n,
        "awT": awT_n, "ab": ab_n, "cwT": cwT_n, "cb": cb_n,
    }
    in_maps = []
    for i in range(N_CORES):
        keys_i = dnd_keys[i * NM:(i + 1) * NM]
        m = dict(shared)
        m["keysT"] = np.ascontiguousarray(
            keys_i.T.reshape(4, 128, NM).transpose(1, 0, 2))
        m["kks"] = np.ascontiguousarray(
            (keys_i * keys_i).sum(1).reshape(NCH, 128).T)
        m["vals"] = np.ascontiguousarray(dnd_vals[i * NM:(i + 1) * NM])
        in_maps.append(m)

    trace = bool(os.environ.get("BASS_TRACE"))
    res = run_bass_kernel_spmd(_NC, in_maps, list(range(N_CORES)), trace=trace)
    LAST_RESULT = res
    r = res.results[0]
    action_logits = np.ascontiguousarray(r["o_logitsT"].T)
    value_estimate = np.ascontiguousarray(r["o_valueT"].T)
    h_new = np.ascontiguousarray(r["o_hT"].T)[None]
    c_new = np.ascontiguousarray(r["o_cT"].T)[None]
    feats = np.ascontiguousarray(r["o_featsT"].T)
    return (action_logits, value_estimate, h_new, c_new, feats)


# revision 6
# speedup vs baseline: 1.7535x; 1.5252x over previous
"""Trainium2 Bass kernel for nn_A2C_DND (A2C agent step with DND kNN retrieval).

Sharding: dict_len (NMEM=65536) is sharded across the 8 NeuronCores (8192
rows of dnd_keys/dnd_vals per core).  Every core computes, for all 1024
cues, the unnormalized softmax-weighted partial value sum and the partial
softmax normalizer over its shard; an AllReduce combines them.  The small
encoder/LSTM/head network is replicated on every core (it overlaps the
collective).  All large matmuls run as float32r (full PE rate, ~1e-4 rel
accuracy).  Outputs are computed transposed on-device and transposed back
on the host.

Softmax is computed without max-subtraction: sims = -sqrt(d2) with d2 ~
N(1024, 55) for this problem's input distribution, so exp(sims) is around
1e-14 -- comfortably inside fp32 range.  h0/c0 are zeros per the problem
spec; the device program exploits that (f-gate and c0/h0 terms dropped).
A full numpy fallback handles the (never occurring) nonzero case.
"""

import os

import numpy as np

import concourse.bass as bass
import concourse.mybir as mybir
from concourse import bacc, tile
from concourse.bass_utils import run_bass_kernel_spmd


def _focus_act_tables():
    """Restrict the ACT table-set chooser to two sets so the per-tile
    ln/exp chain and the gate sigmoid/tanh chain each stay on one
    resident table set (one ACT_TABLE_LOAD each instead of per-tile
    reloads).  Set names/order are preserved; we only blank membership
    of the other sets so the placement pass can't pick them."""
    orig = bacc.get_activation_tables

    def patched(arch):
        tabs = orig(arch)
        keep = {"natural_log_exp_and_others", "sigmoid_and_others"}
        return {name: (s if name in keep else set())
                for name, s in tabs.items()}

    bacc.get_activation_tables = patched
    return orig

B, A, H, DKEY, NMEM = 1024, 4, 512, 512, 65536
OBS, E1, E2 = 9, 64, 128
N_CORES = 8
NM = NMEM // N_CORES  # 8192 rows per core
NCH = NM // 128       # 64 nmem chunks per core

F32 = mybir.dt.float32
F32R = mybir.dt.float32r
AF = mybir.ActivationFunctionType
ALU = mybir.AluOpType

LAST_RESULT = None  # BassKernelResults of the last run (for test.py)

_NC = None  # cached compiled Bacc


def _build():
    nc = bacc.Bacc(
        "TRN2", target_bir_lowering=False, debug=False, num_devices=N_CORES
    )

    def din(name, shape):
        return nc.dram_tensor(name, shape, F32, kind="ExternalInput").ap()

    def dout(name, shape):
        return nc.dram_tensor(name, shape, F32, kind="ExternalOutput").ap()

    cueT = din("cueT", [128, 4, B])       # [k_in, kchunk, b]
    ccb = din("ccb", [1, B])              # ||cue||^2 per b
    keysT = din("keysT", [128, 4, NM])    # [k_in, kchunk, n] (per-core shard)
    kks = din("kks", [128, NCH])          # ||key||^2, [n_in, nchunk]
    vals = din("vals", [NM, H])           # per-core shard, natural layout
    obsT = din("obsT", [OBS, B])
    paprT = din("paprT", [5, B])          # [p_action; p_reward] transposed
    w1T = din("w1T", [OBS, E1])
    b1 = din("b1", [E1, 1])
    w2T = din("w2T", [E1, E2])
    b2 = din("b2", [E2, 1])
    wihTA = din("wihTA", [128, 16, 128])  # w_ih.T[:128, used-cols] per z-chunk
    wihTB = din("wihTB", [5, 16, 128])    # w_ih.T[128:133, used-cols]
    bz = din("bz", [128, 16])             # (b_ih + b_hh)[used-cols]
    awT = din("awT", [128, 4, A])
    ab = din("ab", [A, 1])
    cwT = din("cwT", [128, 4, 1])
    cb = din("cb", [1, 1])

    o_logitsT = dout("o_logitsT", [A, B])
    o_valueT = dout("o_valueT", [1, B])
    o_hT = dout("o_hT", [H, B])
    o_cT = dout("o_cT", [H, B])
    o_featsT = dout("o_featsT", [E2, B])

    # collective bounce buffers (DRAM; outputs must be Shared)
    cc_in = [nc.dram_tensor(f"cc_in{s}", [128, 2048], F32).ap() for s in range(2)]
    cc_out = [
        nc.dram_tensor(f"cc_out{s}", [128, 2048], F32, addr_space="Shared").ap()
        for s in range(2)
    ]
    zc_in = nc.dram_tensor("zc_in", [2, 512], F32).ap()
    zc_out = nc.dram_tensor("zc_out", [2, 512], F32, addr_space="Shared").ap()
    zr_dram = nc.dram_tensor("zr_dram", [2, 512], F32).ap()

    groups = [list(range(N_CORES))]

    def bcast(src_ap, parts):
        # partition-broadcast view of a [1, N] DRAM AP
        return bass.AP(
            tensor=src_ap.tensor, offset=src_ap.offset,
            ap=[[0, parts]] + src_ap.ap[1:],
        )

    with tile.TileContext(nc) as tc:
        with (
            tc.tile_pool(name="consts", bufs=1) as consts,
            tc.tile_pool(name="pdot", bufs=3, space="PSUM") as pdot,
            tc.tile_pool(name="pacc", bufs=1, space="PSUM") as pacc,
        ):
            # ---- resident constants ----
            ct = consts.tile([128, 4, B], F32R)
            nc.sync.dma_start(out=ct[:], in_=cueT.bitcast(F32R))
            cct = consts.tile([128, B], F32)
            nc.sync.dma_start(out=cct[:], in_=bcast(ccb, 128))
            kkt = consts.tile([128, NCH], F32)
            nc.sync.dma_start(out=kkt[:], in_=kks[:])
            ones = consts.tile([128, 1], F32R)
            nc.vector.memset(ones.bitcast(F32), 1.0)
            obst = consts.tile([OBS, B], F32R)
            nc.sync.dma_start(out=obst[:], in_=obsT.bitcast(F32R))
            paprt = consts.tile([5, B], F32R)
            nc.sync.dma_start(out=paprt[:], in_=paprT.bitcast(F32R))
            w1t = consts.tile([OBS, E1], F32R)
            nc.sync.dma_start(out=w1t[:], in_=w1T.bitcast(F32R))
            b1t = consts.tile([E1, 1], F32)
            nc.sync.dma_start(out=b1t[:], in_=b1[:])
            w2t = consts.tile([E1, E2], F32R)
            nc.sync.dma_start(out=w2t[:], in_=w2T.bitcast(F32R))
            b2t = consts.tile([E2, 1], F32)
            nc.sync.dma_start(out=b2t[:], in_=b2[:])
            awt = consts.tile([128, 4, A], F32R)
            nc.sync.dma_start(out=awt[:], in_=awT.bitcast(F32R))
            abt = consts.tile([A, 1], F32)
            nc.sync.dma_start(out=abt[:], in_=ab[:])
            cwt = consts.tile([128, 4, 1], F32R)
            nc.sync.dma_start(out=cwt[:], in_=cwT.bitcast(F32R))
            cbt = consts.tile([1, 1], F32)
            nc.sync.dma_start(out=cbt[:], in_=cb[:])

            # ---- encoder MLP (replicated, tiny) ----
            f1 = consts.tile([E1, B], F32R)
            feats = consts.tile([E2, B], F32R)
            for s in range(2):
                sl = slice(s * 512, (s + 1) * 512)
                p1 = pdot.tile([E1, 512], F32, tag="dot")
                nc.tensor.matmul(p1[:], lhsT=w1t[:], rhs=obst[:, sl],
                                 start=True, stop=True)
                nc.scalar.activation(out=f1[:, sl], in_=p1[:], func=AF.Relu,
                                     bias=b1t[:])
                p2 = pdot.tile([E2, 512], F32, tag="dot")
                nc.tensor.matmul(p2[:], lhsT=w2t[:], rhs=f1[:, sl],
                                 start=True, stop=True)
                nc.scalar.activation(out=feats[:, sl], in_=p2[:], func=AF.Relu,
                                     bias=b2t[:])
            nc.sync.dma_start(out=o_featsT[:], in_=feats.bitcast(F32))

            # ---- DND retrieval over the local shard ----
            with (
                tc.tile_pool(name="keys", bufs=1) as keys_pool,
                tc.tile_pool(name="vstream", bufs=4) as vstream,
                tc.tile_pool(name="estream", bufs=4) as estream,
                tc.tile_pool(name="stage", bufs=1) as stage_pool,
            ):
                kt = []
                for i in range(8):
                    t = keys_pool.tile([128, 4, 1024], F32R, tag=f"kt{i}")
                    nc.sync.dma_start(
                        out=t[:],
                        in_=keysT[:, :, i * 1024:(i + 1) * 1024].bitcast(F32R),
                    )
                    kt.append(t)

                for s in range(2):
                    bsl = slice(s * 512, (s + 1) * 512)
                    pv = [pacc.tile([128, 512], F32, tag=f"pv{h}",
                                    name=f"pv{h}") for h in range(4)]
                    za = pacc.tile([1, 512], F32, tag="za")
                    for j in range(NCH):
                        pd = pdot.tile([128, 512], F32, tag="dot")
                        kslab = kt[j // 8]
                        off = (j % 8) * 128
                        for k in range(4):
                            nc.tensor.matmul(
                                pd[:], lhsT=kslab[:, k, off:off + 128],
                                rhs=ct[:, k, bsl],
                                start=(k == 0), stop=(k == 3),
                            )
                        # d2 = -2*dot + ||cue||^2 + ||key||^2
                        # exp(-sqrt(d2)) = exp(-exp(0.5*ln(d2))): ln and exp
                        # share one ACT table set (no per-tile table swaps)
                        nc.vector.scalar_tensor_tensor(
                            out=pd[:], in0=pd[:], scalar=-2.0, in1=cct[:, bsl],
                            op0=ALU.mult, op1=ALU.add,
                        )
                        nc.scalar.activation(out=pd[:], in_=pd[:], func=AF.Ln,
                                             bias=kkt[:, j:j + 1])
                        nc.scalar.activation(out=pd[:], in_=pd[:], func=AF.Exp,
                                             scale=0.5)
                        et = estream.tile([128, 512], F32R, tag="et")
                        nc.scalar.activation(out=et[:], in_=pd[:], func=AF.Exp,
                                             scale=-1.0)
                        vt = vstream.tile([128, 512], F32R, tag="vt")
                        nc.sync.dma_start(
                            out=vt[:],
                            in_=vals[j * 128:(j + 1) * 128, :].bitcast(F32R),
                        )
                        for h in range(4):
                            nc.tensor.matmul(
                                pv[h][:], lhsT=vt[:, h * 128:(h + 1) * 128],
                                rhs=et[:],
                                start=(j == 0), stop=(j == NCH - 1),
                            )
                        nc.tensor.matmul(za[:], lhsT=ones[:], rhs=et[:],
                                         start=(j == 0), stop=(j == NCH - 1))
                    # flush partials -> DRAM -> AllReduce
                    st = stage_pool.tile([128, 2048], F32, tag="mst")
                    for h in range(4):
                        nc.vector.tensor_copy(st[:, h * 512:(h + 1) * 512],
                                              pv[h][:])
                    zst = stage_pool.tile([1, 512], F32, tag="zst")
                    nc.vector.tensor_copy(zst[:], za[:])
                    nc.sync.dma_start(out=cc_in[s][:], in_=st[:])
                    nc.sync.dma_start(out=zc_in[s:s + 1, :], in_=zst[:])
                    nc.gpsimd.collective_compute(
                        "AllReduce", ALU.add, replica_groups=groups,
                        ins=[cc_in[s].opt()], outs=[cc_out[s].opt()],
                    )
                nc.gpsimd.collective_compute(
                    "AllReduce", ALU.add, replica_groups=groups,
                    ins=[zc_in.opt()], outs=[zc_out.opt()],
                )

            # ---- LSTM gates (overlap the collectives) ----
            with tc.tile_pool(name="gates", bufs=1) as gates:
                gfuncs = [AF.Sigmoid, AF.Tanh, AF.Sigmoid, AF.Sigmoid]  # i g o r
                gt = [gates.tile([128, 4, B], F32, tag=f"gate{g}",
                                 name=f"gate{g}") for g in range(4)]
                with tc.tile_pool(name="wpool", bufs=1) as wpool:
                    wiha = wpool.tile([128, 16, 128], F32R)
                    nc.sync.dma_start(out=wiha[:], in_=wihTA.bitcast(F32R))
                    wihb = wpool.tile([5, 16, 128], F32R)
                    nc.sync.dma_start(out=wihb[:], in_=wihTB.bitcast(F32R))
                    bzt = wpool.tile([128, 16], F32)
                    nc.sync.dma_start(out=bzt[:], in_=bz[:])

                    for u in range(16):
                        gi, hc = u // 4, u % 4
                        for s in range(2):
                            bsl = slice(s * 512, (s + 1) * 512)
                            pz = pdot.tile([128, 512], F32, tag="dot")
                            nc.tensor.matmul(pz[:], lhsT=wiha[:, u, :],
                                             rhs=feats[:, bsl],
                                             start=True, stop=False)
                            nc.tensor.matmul(pz[:], lhsT=wihb[:, u, :],
                                             rhs=paprt[:, bsl],
                                             start=False, stop=True)
                            nc.scalar.activation(out=gt[gi][:, hc, bsl],
                                                 in_=pz[:], func=gfuncs[gi],
                                                 bias=bzt[:, u:u + 1])

                # ---- combine: m_t = m_sum / z_sum ; LSTM cell ; heads ----
                tmp_cm = tc.tile_pool(name="tmp", bufs=2)
                tmp = tmp_cm.__enter__()
                m_sum = gates.tile([128, 4, B], F32)
                for s in range(2):
                    for h in range(4):
                        nc.sync.dma_start(
                            out=m_sum[:, h, s * 512:(s + 1) * 512],
                            in_=cc_out[s][:, h * 512:(h + 1) * 512],
                        )
                zt = gates.tile([2, 512], F32)
                nc.sync.dma_start(out=zt[:], in_=zc_out[:])
                zr = gates.tile([2, 512], F32)
                nc.vector.reciprocal(zr[:], zt[:])
                nc.sync.dma_start(out=zr_dram[:], in_=zr[:])
                rb = gates.tile([128, B], F32)
                for s in range(2):
                    src = zr_dram[s:s + 1, :]
                    nc.sync.dma_start(out=rb[:, s * 512:(s + 1) * 512],
                                      in_=bcast(src, 128))

                hTr = gates.tile([128, 4, B], F32R)
                ti, tg, to, tr = gt
                for h in range(4):
                    mn = m_sum[:, h, :]
                    nc.vector.tensor_mul(mn, mn, rb[:])        # normalize
                    nc.vector.tensor_mul(mn, mn, tr[:, h, :])  # r * m_t
                    t1 = tmp.tile([128, B], F32, tag="t1")
                    nc.vector.tensor_mul(t1[:], ti[:, h, :], tg[:, h, :])
                    nc.vector.tensor_add(t1[:], t1[:], mn)     # c_new chunk
                    nc.sync.dma_start(out=o_cT[h * 128:(h + 1) * 128, :],
                                      in_=t1[:])
                    th = tmp.tile([128, B], F32, tag="th")
                    nc.scalar.activation(out=th[:], in_=t1[:], func=AF.Tanh)
                    nc.vector.tensor_mul(th[:], to[:, h, :], th[:])  # h_new
                    nc.sync.dma_start(out=o_hT[h * 128:(h + 1) * 128, :],
                                      in_=th[:])
                    nc.scalar.activation(out=hTr[:, h, :], in_=th[:],
                                         func=AF.Copy)

                lg = gates.tile([A, B], F32)
                vl = gates.tile([1, B], F32)
                for s in range(2):
                    bsl = slice(s * 512, (s + 1) * 512)
                    pl = pdot.tile([A, 512], F32, tag="dot")
                    for h in range(4):
                        nc.tensor.matmul(pl[:], lhsT=awt[:, h, :],
                                         rhs=hTr[:, h, bsl],
                                         start=(h == 0), stop=(h == 3))
                    nc.scalar.activation(out=lg[:, bsl], in_=pl[:],
                                         func=AF.Identity, bias=abt[:])
                    pv2 = pdot.tile([1, 512], F32, tag="dot")
                    for h in range(4):
                        nc.tensor.matmul(pv2[:], lhsT=cwt[:, h, :],
                                         rhs=hTr[:, h, bsl],
                                         start=(h == 0), stop=(h == 3))
                    nc.scalar.activation(out=vl[:, bsl], in_=pv2[:],
                                         func=AF.Identity, bias=cbt[:])
                nc.sync.dma_start(out=o_logitsT[:], in_=lg[:])
                nc.sync.dma_start(out=o_valueT[:], in_=vl[:])
                tmp_cm.__exit__(None, None, None)

    _orig_tables = _focus_act_tables()
    try:
        nc.compile()
    finally:
        bacc.get_activation_tables = _orig_tables
    return nc


def _np_reference(obs, p_action, p_reward, h0, c0, cue,
                  enc_w1, enc_b1, enc_w2, enc_b2,
                  w_ih, w_hh, b_ih, b_hh,
                  actor_w, actor_b, critic_w, critic_b,
                  dnd_keys, dnd_vals):
    # exact numpy fallback (used only if h0/c0 are nonzero)
    def sigmoid(x):
        return 1.0 / (1.0 + np.exp(-x))

    relu = lambda x: np.maximum(x, 0.0)
    feats = relu(relu(obs @ enc_w1.T + enc_b1) @ enc_w2.T + enc_b2)
    x_t = np.concatenate([feats, p_action, p_reward], axis=-1)
    d2 = ((cue * cue).sum(-1, keepdims=True) - 2.0 * (cue @ dnd_keys.T)
          + (dnd_keys * dnd_keys).sum(-1))
    sims = -np.sqrt(np.maximum(d2, 1e-12))
    sims = sims - sims.max(-1, keepdims=True)
    w = np.exp(sims)
    w /= w.sum(-1, keepdims=True)
    m_t = w @ dnd_vals
    h, c = h0[0], c0[0]
    z = x_t @ w_ih.T + h @ w_hh.T + b_ih + b_hh
    i_g = sigmoid(z[:, 0 * H:1 * H])
    f_g = sigmoid(z[:, 1 * H:2 * H])
    g_g = np.tanh(z[:, 2 * H:3 * H])
    o_g = sigmoid(z[:, 3 * H:4 * H])
    r_g = sigmoid(z[:, 4 * H:5 * H])
    c_new = f_g * c + i_g * g_g + r_g * m_t
    h_new = o_g * np.tanh(c_new)
    action_logits = h_new @ actor_w.T + actor_b
    value_estimate = h_new @ critic_w.T + critic_b
    return (action_logits.astype(np.float32),
            value_estimate.astype(np.float32),
            h_new[None].astype(np.float32), c_new[None].astype(np.float32),
            feats.astype(np.float32))


def kernel(obs, p_action, p_reward, h0, c0, cue,
           enc_w1, enc_b1, enc_w2, enc_b2,
           w_ih, w_hh, b_ih, b_hh,
           actor_w, actor_b, critic_w, critic_b,
           dnd_keys, dnd_vals):
    global _NC, LAST_RESULT
    f = lambda x: np.ascontiguousarray(np.asarray(x, dtype=np.float32))
    obs, p_action, p_reward = f(obs), f(p_action), f(p_reward)
    h0, c0, cue = f(h0), f(c0), f(cue)
    enc_w1, enc_b1, enc_w2, enc_b2 = f(enc_w1), f(enc_b1), f(enc_w2), f(enc_b2)
    w_ih, w_hh, b_ih, b_hh = f(w_ih), f(w_hh), f(b_ih), f(b_hh)
    actor_w, actor_b, critic_w, critic_b = f(actor_w), f(actor_b), f(critic_w), f(critic_b)
    dnd_keys, dnd_vals = f(dnd_keys), f(dnd_vals)

    if np.any(h0) or np.any(c0):
        return _np_reference(obs, p_action, p_reward, h0, c0, cue,
                             enc_w1, enc_b1, enc_w2, enc_b2,
                             w_ih, w_hh, b_ih, b_hh,
                             actor_w, actor_b, critic_w, critic_b,
                             dnd_keys, dnd_vals)

    if _NC is None:
        _NC = _build()

    cueT_n = np.ascontiguousarray(cue.T.reshape(4, 128, B).transpose(1, 0, 2))
    ccb_n = np.ascontiguousarray((cue * cue).sum(1)[None, :])
    obsT_n = np.ascontiguousarray(obs.T)
    paprT_n = np.ascontiguousarray(
        np.concatenate([p_action.T, p_reward.T], axis=0))
    w1T_n = np.ascontiguousarray(enc_w1.T)
    b1_n = np.ascontiguousarray(enc_b1[:, None])
    w2T_n = np.ascontiguousarray(enc_w2.T)
    b2_n = np.ascontiguousarray(enc_b2[:, None])
    cols = np.r_[0:512, 1024:2560]
    wihT = w_ih.T
    wihTA_n = np.ascontiguousarray(wihT[:128, cols].reshape(128, 16, 128))
    wihTB_n = np.ascontiguousarray(wihT[128:133, cols].reshape(5, 16, 128))
    bz_n = np.ascontiguousarray((b_ih + b_hh)[cols].reshape(16, 128).T)
    awT_n = np.ascontiguousarray(actor_w.T.reshape(4, 128, A).transpose(1, 0, 2))
    ab_n = np.ascontiguousarray(actor_b[:, None])
    cwT_n = np.ascontiguousarray(critic_w.T.reshape(4, 128, 1).transpose(1, 0, 2))
    cb_n = np.ascontiguousarray(critic_b[:, None])

    shared = {
        "cueT": cueT_n, "ccb": ccb_n, "obsT": obsT_n, "paprT": paprT_n,
        "w1T": w1T_n, "b1": b1_n, "w2T": w2T_n, "b2": b2_n,
        "wihTA": wihTA_n, "wihTB": wihTB_n, "bz": bz_n,
        "awT": awT_n, "ab": ab_n, "cwT": cwT_n, "cb": cb_n,
    }
    in_maps = []
    for i in range(N_CORES):
        keys_i = dnd_keys[i * NM:(i + 1) * NM]
        m = dict(shared)
        m["keysT"] = np.ascontiguousarray(
            keys_i.T.reshape(4, 128, NM).transpose(1, 0, 2))
        m["kks"] = np.ascontiguousarray(
            (keys_i * keys_i).sum(1).reshape(NCH, 128).T)
        m["vals"] = np.ascontiguousarray(dnd_vals[i * NM:(i + 1) * NM])
        in_maps.append(m)

    trace = bool(os.environ.get("BASS_TRACE"))
    res = run_bass_kernel_spmd(_NC, in_maps, list(range(N_CORES)), trace=trace)
    LAST_RESULT = res
    r = res.results[0]
    action_logits = np.ascontiguousarray(r["o_logitsT"].T)
    value_estimate = np.ascontiguousarray(r["o_valueT"].T)
    h_new = np.ascontiguousarray(r["o_hT"].T)[None]
    c_new = np.ascontiguousarray(r["o_cT"].T)[None]
    feats = np.ascontiguousarray(r["o_featsT"].T)
    return (action_logits, value_estimate, h_new, c_new, feats)
